# revision 69
# baseline (speedup 1.0000x reference)
"""Trainium2 Bass kernel for nn_CrossAttentionLayer (B=8, N=2048, Q=256, D=1024, H=16).

Data-parallel over batch: 1 sample per NeuronCore, 8 cores, no collectives.

Math identities (host-side folds):
  - b_k dropped (constant shift along the softmax axis)
  - b_v folded through out_proj: bout_eff = b_out + W_o b_v (softmax rows sum to 1)
  - b_q and the 1/sqrt(HD) scale folded into the Q-projection eviction
  - exp computed as exp(s - 3): the e^-3 cancels in softmax normalization and
    keeps probs inside fp8-e4m3 normal range (max score ~5.5 -> e^2.5 ~ 12)

Precision plan (rel-err gate is 2e-2; residual dominates the output):
  - K/V/Q/out projections in fp8 e4m3 (weights host-scaled x32, unscaled at
    psum eviction); all projections use DoubleRow (2 k-tiles per matmul)
  - scores in bf16 with per-head-pair ROW TILING: head 2p on PE rows 0-63,
    head 2p+1 on rows 64-127; matmuls of the two row-groups are interleaved
    at single-mm granularity so the PE runs them concurrently (row_grp
    tiling: Dstart ~4ns) and LDWEIGHTS of one group hides under the other's
    streaming.
  - probs in fp8 from ACT exp; attn@V in fp8 DoubleRow with a ones column
    appended to V (row 64 of the psum = softmax denominator, free)
  - normalization: batched reciprocal of denominators + PE broadcast (x32 to
    re-center fp8) + one DVE mul per pair
  - output evicted as bf16 with scale+residual fused in one DVE
    scalar_tensor_tensor (psum*1/1024 + resid), halving the out-DMA

Perf notes (measured): the PE sustains ~215ns per 512-col DR matmul and
~110ns per 256-col matmul on this part, so the kernel is PE-bound at
~125-130us of tensor work; the ACT exp stream (64 x ~1.1us = ~70us) rides
under it. Weight loads use DoubleRowSwInterleave with host-interleaved
contiguous weight buffers (wq/wk/srcI) so LDWEIGHTS reads are contiguous.
The schedule is WINDOW-MAJOR: all 8 pairs' window-0 score chunks run
first (they need only the first-priority head DMA bundle + src w0 + wk),
kproj is emitted just-in-time per (pair, window), V-proj units pace
windows 1-2, and in window 3 each pair's attnv/norm chases one/two pairs
behind the score stream so only A7/N7/out-proj trail the last matmul.
First exp fires ~13us in; measured ~140-143us cool, higher when the part
is thermally downclocked by back-to-back runs.
"""

import numpy as np
import ml_dtypes
from contextlib import ExitStack

import concourse.bass as bass
import concourse.mybir as mybir
import concourse.tile as tile
from concourse import bacc
from concourse.bass_utils import run_bass_kernel_spmd

F32 = mybir.dt.float32
BF16 = mybir.dt.bfloat16
FP8 = mybir.dt.float8e4
AF = mybir.ActivationFunctionType
ALU = mybir.AluOpType
DR = mybir.MatmulPerfMode.DoubleRow
DRSW = mybir.MatmulPerfMode.DoubleRowSwInterleave

NP_FP8 = ml_dtypes.float8_e4m3
NP_BF16 = ml_dtypes.bfloat16

B, N, Q, D, H = 8, 2048, 256, 1024, 16
HD = D // H            # 64
KT = D // 128          # 8 contraction tiles
MT = D // 128          # 8 output tiles
NT = N // 128          # 16 token tiles
NW = N // 512          # 4 token windows (DMA + Kproj chunking)
PAIRS = H // 2         # 8 head pairs
WS = 32.0              # host weight pre-scale for fp8
N_CORES = 8


def build():
    nc = bacc.Bacc(None, target_bir_lowering=False)
    src8 = nc.declare_dram_parameter("src8", [NW, 128, KT, 512], FP8, isOutput=False)
    srcI8 = nc.declare_dram_parameter("srcI8", [128, KT // 2, NT, 256], FP8, isOutput=False)
    # head bundle: qry + bq bytes + wq m0-1 + wk m0 (one first-priority DMA)
    HB = KT * Q + 4 * MT + 2 * KT // 2 * 256 + KT // 2 * 256
    qb8 = nc.declare_dram_parameter("qb8", [128, HB], FP8, isOutput=False)
    wq8 = nc.declare_dram_parameter("wq8", [128, MT - 2, KT // 2, 256], FP8, isOutput=False)
    wk8 = nc.declare_dram_parameter("wk8", [128, MT - 1, KT // 2, 256], FP8, isOutput=False)
    # tail weights bundle: wv + wo, each [KT, D] fp8
    wvo8 = nc.declare_dram_parameter("wvo8", [128, 2, KT, D], FP8, isOutput=False)
    resid = nc.declare_dram_parameter("resid", [128, Q // 128, D], BF16, isOutput=False)
    out = nc.declare_dram_parameter("out", [Q, D], BF16, isOutput=True)

    with tile.TileContext(nc) as tc, ExitStack() as ctx:
        # PSUM budget (8 banks): proj 2 + sc 2x2 + pso 2
        proj_ps = ctx.enter_context(tc.tile_pool(name="projps", bufs=2, space="PSUM"))
        sc_ps = ctx.enter_context(tc.tile_pool(name="scps", bufs=2, space="PSUM"))
        pso_ps = ctx.enter_context(tc.tile_pool(name="psops", bufs=2, space="PSUM"))

        big = ctx.enter_context(tc.tile_pool(name="big", bufs=1))

        rb_pool = ctx.enter_context(tc.tile_pool(name="rbp", bufs=2))
        out_pool = ctx.enter_context(tc.tile_pool(name="outp", bufs=2))

        src_sb = big.tile([128, KT, NW, 512], FP8, tag="src")
        srcI_sb = big.tile([128, KT // 2, NT, 256], FP8, tag="srcI")
        wk_sb = big.tile([128, MT - 1, KT // 2, 256], FP8, tag="wk")
        qb_sb = big.tile([128, KT * Q + 4 * MT + 3 * KT // 2 * 256], FP8, tag="qb")
        wq_sb = big.tile([128, MT - 2, KT // 2, 256], FP8, tag="wq")
        wvo_sb = big.tile([128, 2, KT, D], FP8, tag="wvo")
        qry_sb = qb_sb[:, 0:KT * Q].rearrange("p (k q) -> p k q", k=KT)
        bq_sb = qb_sb[:, KT * Q:KT * Q + 4 * MT].bitcast(F32)
        _o1 = KT * Q + 4 * MT
        wq01_sb = qb_sb[:, _o1:_o1 + KT * 256].rearrange(
            "p (m k f) -> p m k f", m=2, k=KT // 2)
        wk0_sb = qb_sb[:, _o1 + KT * 256:].rearrange(
            "p (k f) -> p k f", k=KT // 2)
        wv_sb = wvo_sb[:, 0]
        wo_sb = wvo_sb[:, 1]
        kt_sb = big.tile([128, MT, 2, 512], BF16, tag="kt")
        qt_sb = big.tile([128, MT, Q], BF16, tag="qt")
        v_sb = big.tile([128, NT, H, 65], FP8, tag="v")
        den_sb = big.tile([1, H, Q], BF16, tag="den")
        e32_sb = big.tile([1, 64], BF16, tag="e32")
        resid_sb = big.tile([128, Q // 128, D], BF16, tag="res")
        warm_sb = big.tile([16, 256], BF16, tag="warm")
        warmo_sb = big.tile([16, 16], F32, tag="warmo")
        nb_sb = big.tile([128, 1], F32, tag="negbias")

        # ---- init: memsets (gpsimd), ACT exp-table warm, PE HAM warm ----
        nc.gpsimd.memset(warm_sb, 0.0)
        nc.vector.memset(warmo_sb, 0.0)
        nc.gpsimd.memset(v_sb[:, :, :, 64:65], 1.0)   # denominator ones column
        nc.gpsimd.memset(nb_sb, -3.0)                 # exp re-centering bias
        nc.vector.memset(e32_sb, 1.0 / WS)  # denb = den/32 -> rb = 32/den
        # load the exp table set early (hides the ~2.7us ACT_TABLE_LOAD)
        nc.scalar.activation(out=warmo_sb[0:1, :], in_=warm_sb[0:1, 0:16], func=AF.Exp)
        # dummy matmuls to trip the PE HAM un-throttle and bridge the
        # head-bundle DMA wait (~3us) without delaying qproj
        for i in range(8):
            wp = proj_ps.tile([16, 256], F32, tag="proj", name=f"warm{i}")
            nc.tensor.matmul(wp[:], lhsT=warm_sb[:, 0:16], rhs=warm_sb[:], start=True, stop=True)
        # ---- DMA loads (sync queue), priority order ----
        nc.sync.dma_start(out=qb_sb, in_=qb8[:])
        nc.sync.dma_start(out=src_sb[:, :, 0, :], in_=src8[0])
        nc.sync.dma_start(out=wk_sb, in_=wk8[:])
        nc.sync.dma_start(out=wq_sb, in_=wq8[:])
        nc.sync.dma_start(out=src_sb[:, :, 1, :], in_=src8[1])
        nc.sync.dma_start(out=srcI_sb, in_=srcI8[:])
        nc.sync.dma_start(out=wvo_sb, in_=wvo8[:])
        nc.sync.dma_start(out=src_sb[:, :, 2, :], in_=src8[2])
        nc.sync.dma_start(out=src_sb[:, :, 3, :], in_=src8[3])
        nc.sync.dma_start(out=resid_sb, in_=resid[:])

        # ---- Q projection (fp8 DoubleRow): qt = (psum/(WS*8)) + b_q/8 ----
        def emit_qproj(m):
            qp = proj_ps.tile([128, Q], F32, tag="proj", name=f"qp{m}")
            for k in range(KT // 2):
                nc.tensor.matmul(
                    qp[:],
                    lhsT=(wq01_sb[:, m, k, :] if m < 2 else wq_sb[:, m - 2, k, :]),
                    rhs=qry_sb[:, 2 * k:2 * k + 2, :],
                    start=(k == 0), stop=(k == KT // 2 - 1), perf_mode=DRSW,
                )
            nc.vector.tensor_scalar(
                out=qt_sb[:, m, :], in0=qp[:],
                scalar1=1.0 / (WS * 8.0), scalar2=bq_sb[:, m:m + 1],
                op0=ALU.mult, op1=ALU.add,
            )

        # ---- K projection (fp8 DoubleRow): kT[dout, tok] = W_k @ src^T ----
        def emit_kproj(m, w):
            kp = proj_ps.tile([128, 512], F32, tag="proj", name=f"kp{m}_{w}")
            for k in range(KT // 2):
                nc.tensor.matmul(
                    kp[:],
                    lhsT=(wk0_sb[:, k, :] if m == 0 else wk_sb[:, m - 1, k, :]),
                    rhs=src_sb[:, 2 * k:2 * k + 2, w, :],
                    start=(k == 0), stop=(k == KT // 2 - 1), perf_mode=DRSW,
                )
            nc.vector.tensor_scalar_mul(
                out=kt_sb[:, m, w % 2], in0=kp[:], scalar1=1.0 / WS
            )

        # ---- V projection (fp8 DoubleRow): v[tok, h, hd] = src @ W_v^T ----
        def emit_vproj(c, t):
            vp = proj_ps.tile([128, 512], F32, tag="proj", name=f"vp{c}_{t}")
            for k in range(KT // 2):
                nc.tensor.matmul(
                    vp[:],
                    lhsT=srcI_sb[:, k, t, :],
                    rhs=wv_sb[:, 2 * k:2 * k + 2, c * 512:(c + 1) * 512],
                    start=(k == 0), stop=(k == KT // 2 - 1), perf_mode=DRSW,
                )
            nc.vector.tensor_scalar_mul(
                out=v_sb[:, t, c * 8:(c + 1) * 8, 0:64],
                in0=vp[:].rearrange("p (h d) -> p h d", h=8),
                scalar1=1.0 / WS,
            )

        # ---- attention ----
        # window-major: all 16 heads' exp planes are live simultaneously
        expt_sb = big.tile([128, H, NT, Q], FP8, tag="expt")
        expt = {(p, par): expt_sb[:, 2 * p + par]
                for p in range(PAIRS) for par in range(2)}

        def emit_score_pair(p, c):
            # 8 score matmuls for BOTH heads of pair p, n-tiles 4c..4c+3,
            # interleaved mm-by-mm across PE row groups (0-63 / 64-127) so
            # the two streams run concurrently; 2 exps chase them on ACT.
            sc = [
                sc_ps.tile([128, 4, Q], F32, tag="sc", name=f"sc{p}_{c}_{par}")
                for par in range(2)
            ]
            for j in range(4):
                nt = 4 * c + j
                for par in range(2):
                    po = par * 64
                    nc.tensor.matmul(
                        sc[par][:, j, :],
                        lhsT=kt_sb[po:po + 64, p, c % 2, j * 128:(j + 1) * 128],
                        rhs=qt_sb[po:po + 64, p, :],
                        start=True, stop=True,
                    )
            for par in range(2):
                nc.scalar.activation(
                    out=expt[(p, par)][:, 4 * c:4 * c + 4, :], in_=sc[par][:],
                    func=AF.Exp, bias=nb_sb[:],
                )

        def emit_attnv(p):
            for par in range(2):
                h = 2 * p + par
                pso = pso_ps.tile([65, Q], F32, tag="pso", name=f"pso{h}")
                for tt in range(NT // 2):
                    nc.tensor.matmul(
                        pso[:],
                        lhsT=v_sb[:, 2 * tt:2 * tt + 2, h, 0:65],
                        rhs=expt[(p, par)][:, 2 * tt:2 * tt + 2, :],
                        start=(tt == 0), stop=(tt == NT // 2 - 1), perf_mode=DR,
                    )
                nc.vector.tensor_copy(ao_sb[par * 64:par * 64 + 64, p, :], pso[0:64, :])
                nc.vector.tensor_copy(den_sb[:, h, :], pso[64:65, :])

        def emit_norm(p):
            denb = proj_ps.tile([128, Q], F32, tag="proj", name=f"denb{p}")
            for par in range(2):
                nc.tensor.matmul(
                    denb[par * 64:par * 64 + 64, :], lhsT=e32_sb[:],
                    rhs=den_sb[:, 2 * p + par, :], start=True, stop=True,
                )
            rb = rb_pool.tile([128, Q], F32, tag="rb", name=f"rb{p}")
            nc.vector.reciprocal_approx_fast(out=rb[:], in_=denb[:])
            nc.vector.tensor_mul(ao8_sb[:, p, :], ao_sb[:, p, :], rb[:])

        ao_sb = big.tile([128, MT, Q], BF16, tag="ao")
        ao8_sb = big.tile([128, MT, Q], FP8, tag="ao8")

        # ---- schedule: window-major scores ----
        # Window 0 (token tiles 0..3) needs only the head bundle + src w0 +
        # wk, so all 8 pairs' first score chunks run while the rest of the
        # DMAs land. qproj(2..7) are emitted just-in-time as their wq slices
        # arrive. V-proj units pace windows 1-2; in window 3 each pair's
        # attnv+norm chases its last score chunk, so only A7/N7/out-proj
        # trail the final score matmuls.
        emit_qproj(0)
        emit_qproj(1)

        from collections import deque
        spacers = deque()
        for t in range(NT):
            spacers.append((0, t))
        for t in range(NT):
            spacers.append((1, t))

        def pop(dq, n):
            for _ in range(min(n, len(dq))):
                c_, t_ = dq.popleft()
                emit_vproj(c_, t_)

        for p in range(PAIRS):
            if p >= 2:
                emit_qproj(p)
            emit_kproj(p, 0)
            emit_score_pair(p, 0)
        for c in range(1, 3):
            for p in range(PAIRS):
                emit_kproj(p, c)
                emit_score_pair(p, c)
                pop(spacers, 2)
        for p in range(PAIRS):
            emit_kproj(p, 3)
            emit_score_pair(p, 3)
            pop(spacers, 2)
            if p >= 1:
                emit_attnv(p - 1)      # exps of pair p-1 are ~6us old: no wait
            if p >= 2:
                emit_norm(p - 2)
        pop(spacers, 99)
        emit_norm(6)
        # pair 7 with per-half norm: par0's reciprocal+multiply run on DVE
        # while par1's attnv matmuls stream, shortening the tail chain
        denb7 = proj_ps.tile([128, Q], F32, tag="proj", name="denb7")
        rb7 = rb_pool.tile([128, Q], F32, tag="rb", name="rb7")
        for par in range(2):
            h = 14 + par
            pso = pso_ps.tile([65, Q], F32, tag="pso", name=f"pso{h}")
            for tt in range(NT // 2):
                nc.tensor.matmul(
                    pso[:],
                    lhsT=v_sb[:, 2 * tt:2 * tt + 2, h, 0:65],
                    rhs=expt[(7, par)][:, 2 * tt:2 * tt + 2, :],
                    start=(tt == 0), stop=(tt == NT // 2 - 1), perf_mode=DR,
                )
            nc.vector.tensor_copy(ao_sb[par * 64:par * 64 + 64, 7, :], pso[0:64, :])
            nc.vector.tensor_copy(den_sb[:, h, :], pso[64:65, :])
            po = par * 64
            nc.tensor.matmul(
                denb7[po:po + 64, :], lhsT=e32_sb[:],
                rhs=den_sb[:, h, :], start=True, stop=True,
            )
            nc.vector.reciprocal_approx_fast(
                out=rb7[po:po + 64, :], in_=denb7[po:po + 64, :])
            nc.vector.tensor_mul(
                ao8_sb[po:po + 64, 7, :], ao_sb[po:po + 64, 7, :],
                rb7[po:po + 64, :])

        # ---- out projection (fp8 DoubleRow) + fused scale+residual (DVE) ----
        for qt in range(Q // 128):
            for cc in range(2):
                op = proj_ps.tile([128, 512], F32, tag="proj", name=f"op{qt}_{cc}")
                for m in range(MT // 2):
                    nc.tensor.matmul(
                        op[:],
                        lhsT=ao8_sb[:, 2 * m:2 * m + 2, qt * 128:(qt + 1) * 128],
                        rhs=wo_sb[:, 2 * m:2 * m + 2, cc * 512:(cc + 1) * 512],
                        start=(m == 0), stop=(m == MT // 2 - 1), perf_mode=DR,
                    )
                of = out_pool.tile([128, 512], BF16, tag="of", name=f"of{qt}_{cc}")
                nc.vector.scalar_tensor_tensor(
                    out=of[:], in0=op[:], scalar=1.0 / (WS * WS),
                    in1=resid_sb[:, qt, cc * 512:(cc + 1) * 512],
                    op0=ALU.mult, op1=ALU.add,
                )
                eng = nc.sync if cc == 0 else nc.scalar
                eng.dma_start(
                    out=out[qt * 128:(qt + 1) * 128, cc * 512:(cc + 1) * 512], in_=of
                )

    nc.finalize()
    return nc


_NC_CACHE = {}


def _get_nc():
    if "nc" not in _NC_CACHE:
        _NC_CACHE["nc"] = build()
    return _NC_CACHE["nc"]


def _fp8(x):
    return np.clip(x, -240.0, 240.0).astype(NP_FP8)


def make_in_maps(sources, queries, w_in, b_in, w_out, b_out):
    sources = np.asarray(sources, dtype=np.float32)
    queries = np.asarray(queries, dtype=np.float32)
    w_in = np.asarray(w_in, dtype=np.float32)
    b_in = np.asarray(b_in, dtype=np.float32)
    w_out = np.asarray(w_out, dtype=np.float32)
    b_out = np.asarray(b_out, dtype=np.float32)

    w_q, w_k, w_v = w_in[0:D], w_in[D:2 * D], w_in[2 * D:3 * D]
    b_q, b_v = b_in[0:D], b_in[2 * D:3 * D]
    bout_eff = b_out + w_out @ b_v

    def wprep(w):  # [dout, din] -> fp8 [128, KT, D] p-major of (w.T * WS)
        wt = np.ascontiguousarray(w.T) * WS
        return _fp8(wt.reshape(KT, 128, D).transpose(1, 0, 2))

    def swi(w8):
        # [128, KT, D] -> [128, KT//2, MT, 256] sw-interleaved weight pairs:
        # slot 2*i+b of (kk, m) holds w8[p, 2*kk+b, m*128 + 127 - i]
        a = w8.reshape(128, KT // 2, 2, MT, 128)[:, :, :, :, ::-1]
        return np.ascontiguousarray(a.transpose(0, 1, 3, 4, 2).reshape(
            128, KT // 2, MT, 256))

    def m_major(w8):  # [128, KT//2, MT, 256] -> [128, MT, KT//2, 256]
        return np.ascontiguousarray(w8.transpose(0, 2, 1, 3))

    wk8_full = m_major(swi(wprep(w_k)))
    wq8_full = m_major(swi(wprep(w_q)))
    wk8 = np.ascontiguousarray(wk8_full[:, 1:])
    wq8 = np.ascontiguousarray(wq8_full[:, 2:])
    wvo8 = np.stack([wprep(w_v), wprep(w_out)], axis=1)
    # bq as raw bytes: [128, 4*MT] fp8-typed view of [128, MT] f32
    bq_bytes = np.ascontiguousarray(
        (b_q / 8.0).reshape(MT, 128).T).view(NP_FP8).reshape(128, 4 * MT)

    in_maps = []
    for b in range(B):
        st = sources[b].T  # [D, N]
        s8 = _fp8(st.reshape(KT, 128, NW, 512).transpose(2, 1, 0, 3))
        # contiguous sw-interleaved copy for vproj weights:
        # slot 2*i+b of (kk, t) = st[(2*kk+b)*128 + p, t*128 + 127 - i]
        sI = st.reshape(KT // 2, 2, 128, NT, 128)[:, :, :, :, ::-1]
        srcI8 = _fp8(np.ascontiguousarray(
            sI.transpose(2, 0, 3, 4, 1).reshape(128, KT // 2, NT, 256)))
        qt = queries[b].T  # [D, Q]
        qry8 = _fp8(qt.reshape(KT, 128, Q).transpose(1, 0, 2))
        qb8 = np.concatenate(
            [qry8.reshape(128, -1), bq_bytes,
             wq8_full[:, 0:2].reshape(128, -1), wk8_full[:, 0:1].reshape(128, -1)],
            axis=1)
        res = (queries[b] + bout_eff[None, :]).reshape(Q // 128, 128, D).transpose(1, 0, 2).astype(NP_BF16)
        in_maps.append({
            "src8": s8, "srcI8": srcI8, "qb8": qb8, "wq8": wq8,
            "wk8": wk8, "wvo8": wvo8, "resid": res,
        })
    return in_maps


def kernel(sources, queries, w_in, b_in, w_out, b_out, _trace=False):
    nc = _get_nc()
    in_maps = make_in_maps(sources, queries, w_in, b_in, w_out, b_out)
    res = run_bass_kernel_spmd(nc, in_maps, core_ids=list(range(N_CORES)), trace=_trace)
    out = np.stack(
        [res.results[b]["out"].astype(np.float32) for b in range(B)], axis=0
    )
    if _trace:
        kernel.last_exec_time_ns = res.exec_time_ns
        kernel.last_results = res
    return out
